# revision 23
# baseline (speedup 1.0000x reference)
"""Trainium2 Bass kernel for the Griffin-style gated linear recurrence.

Model (matching the jax reference, including its chunked-scan numerics):
    a = sigmoid(x @ Wa.T + decay_bias)
    i = sigmoid(x @ Wi.T)
    v = x @ Wv.T
    w = sqrt(max(1 - a*a, 1e-8)) * i * v
    chunked scan (chunk=64), algebraically equal to
    h[t] = a[t]*h[t-1] + g[t]*w[t],  g[t] = min(1, cd[t]*1e10),
    cd = within-chunk running product of a (reset every 64 steps).

Sharding: 4 batches x 2 channel-halves = 8 cores, no communication.

Matmul layout (per core): channels on partitions, time on the free axis.
The 576 projection columns (3 gates x 192 channels) are packed into 5
stationary tiles per k-tile instead of 6:
    z0 = a[0:128]   z1 = i[0:128]   z2 = v[0:128]
    z3 = [a[128:192] ; i[128:192]]  (64+64 partitions, one sigmoid with a
                                     [bias_hi ; 0] per-partition bias)
    z4 = v[128:192]                  (64 partitions)
which cuts tensor-engine time by 1/6 (matmul cost is N-cycles per
instruction regardless of M).  x and weights stream in bf16; PSUM
accumulates fp32.  A dozen warm-up matmuls on a zeroed scratch tile
bring the PE out of its low p-state (0.65->2.4GHz takes ~3us of
continuous busy) while the first x block is still in flight.

Pipeline structure (learned from several traced revisions):
  * Matmuls/PSUM work per 512-column block, but ALL trailing pointwise
    runs once per 1024-column block pair -- per-instruction overheads on
    DVE/Pool/Act were measured at 0.4-1.3us, so halving the op count is
    worth more than any engine rebalancing.
  * PSUM z0/z3 (bufs=1) are drained by DVE copies into SBUF pair tiles
    (GPSIMD and DMA cannot read PSUM) and the sigmoids read those
    copies.  Draining via the sigmoids directly made PE stall up to
    8.4us per block: the Act queue sits behind 1.28us activation-table
    loads (sigmoid and sqrt never share a table) and a stalled PE also
    drops back to half clock for ~3us.
  * z1/z2/z4 are double-buffered (2 + 3*2 = 8 PSUM banks, the warm-up
    sharing a z2 slot) and consumed directly at 512 width (Act ii
    sigmoid, DVE u-muls).
  * g = min(cd*1e10, 1) runs on Act as t=Relu(1-1e10*cd); g=1-t (relu
    and copy live in EVERY act table; tensor_scalar was 7.5us/op on
    Pool's Q7 path and the DVE needs the slack for the scans).
  * cd is ONE masked scan per group pair, with the chunk-start masking
    done in the PRE-activation domain: tiny strided sigmoids capture the
    true chunk-start a values (a_s), the chunk-start columns of the
    drained z are memset to -1e9, and the main sigmoids then directly
    produce a with zeroed chunk starts -- the scan self-resets, and no
    full-width a_m copy exists on any engine or on the critical chain.
    Strided fix-ups patch m at those columns and restore true a for the
    h scan afterwards.
  * The decay path (a, m, cd, h, scan operands) stays fp32: sqrt(1-a^2)
    is cancellation-sensitive for slow channels, and a bf16 scan operand
    was measured to double scan time.  The iv path runs bf16.
  * Moving the squares or the gw muls to Pool was measured SLOWER
    (139.9us vs 114.5us): Pool's Q7 tensor_tensor (~3.2us per 1024-wide
    op) sits on the gw->h critical chain.

z3 holds a_hi/i_hi on different partitions, so one cross-partition
SBUF->SBUF DMA per pair realigns i_hi with v_hi (engines are
lane-locked; only DMA can move data across partitions).
"""

import sys

if "/opt/trn_rl_repo" not in sys.path:
    sys.path.insert(0, "/opt/trn_rl_repo")

from contextlib import ExitStack

import numpy as np
import ml_dtypes

from concourse import bacc, bass, mybir, tile
from concourse.bass_utils import run_bass_kernel_spmd

B, S = 4, 4096
DM, DR = 1024, 384
DC = DR // 2          # channels per core
CH = 64               # scan chunk size
SB = 512              # matmul/PSUM block
PW = 2 * SB           # pointwise pair width
NB = S // SB
KT = DM // 128        # contraction tiles

F32 = mybir.dt.float32
BF16 = mybir.dt.bfloat16
AFT = mybir.ActivationFunctionType
OP = mybir.AluOpType

# column ranges of the 5 packed stationary tiles
TCOLS = ((0, 128), (128, 256), (256, 384), (384, 512), (512, 576))

_CACHED_NC = None


def _build_nc():
    nc = bacc.Bacc(trn_type="TRN2")

    xT = nc.dram_tensor("xt", [DM, S], BF16, kind="ExternalInput")
    wc = nc.dram_tensor("wcat", [DM, 576], BF16, kind="ExternalInput")
    bias0 = nc.dram_tensor("bias0", [128, 1], F32, kind="ExternalInput")
    bias3 = nc.dram_tensor("bias3", [128, 1], F32, kind="ExternalInput")
    out = nc.dram_tensor("out", [DC, S], F32, kind="ExternalOutput")

    with tile.TileContext(nc) as tc, ExitStack() as ctx:
        wp = ctx.enter_context(tc.tile_pool(name="wp", bufs=1))
        cp = ctx.enter_context(tc.tile_pool(name="cp", bufs=1))
        xp = ctx.enter_context(tc.tile_pool(name="xp", bufs=2))
        pp = ctx.enter_context(tc.tile_pool(name="pp", bufs=1, space="PSUM"))
        pv = ctx.enter_context(tc.tile_pool(name="pv", bufs=2, space="PSUM"))
        sp = ctx.enter_context(tc.tile_pool(name="sp", bufs=2))
        ap = ctx.enter_context(tc.tile_pool(name="ap", bufs=2))
        hp = ctx.enter_context(tc.tile_pool(name="hp", bufs=2))

        # --- warm-up + constants --------------------------------------
        # x block 0 first: it gates the first real matmul.
        x0 = xp.tile([128, KT, SB], BF16, tag="x")
        nc.sync.dma_start(
            x0[:], xT.rearrange("(k p) s -> p k s", p=128)[:, :, 0:SB])

        # PE warm-up: a zeroed bf16 tile matmul'd into the spare PSUM
        # bank while DMAs land; PE reaches full clock before real work.
        wz = cp.tile([128, SB], BF16, tag="wz")
        nc.gpsimd.memset(wz[:], 0.0)
        zw = pv.tile([128, SB], F32, tag="z2")  # reuses a z2 PSUM slot
        for _ in range(12):
            nc.tensor.matmul(zw[:], wz[:, 0:128], wz[:], start=True,
                             stop=True)

        # per-k weight tiles so the first matmuls gate on 144KB, not 1.15MB
        w_k = []
        for k in range(KT):
            wk = wp.tile([128, 576], BF16, tag=f"wk{k}")
            nc.sync.dma_start(wk[:], wc[k * 128:(k + 1) * 128, :])
            w_k.append(wk)

        b0 = cp.tile([128, 1], F32, tag="b0")
        nc.scalar.dma_start(b0[:], bias0[:, :])
        b3 = cp.tile([128, 1], F32, tag="b3")
        nc.scalar.dma_start(b3[:], bias3[:, :])

        def emit_front(m):
            # SBUF pair tiles that the two matmul sub-blocks fill
            zc0 = sp.tile([128, PW], F32, tag="zc0")
            zc3 = sp.tile([128, PW], F32, tag="zc3")
            u_lo = sp.tile([128, PW], BF16, tag="u_lo")

            z4s = []
            for half in range(2):
                ib = 2 * m + half
                s0 = ib * SB
                hs = slice(half * SB, (half + 1) * SB)

                if ib > 0:  # block 0's x tile was loaded up front
                    x_sb = xp.tile([128, KT, SB], BF16, tag="x")
                    nc.sync.dma_start(
                        x_sb[:],
                        xT.rearrange("(k p) s -> p k s", p=128)
                        [:, :, s0:s0 + SB])
                else:
                    x_sb = x0

                z = []
                for t, (c0, c1) in enumerate(TCOLS):
                    pool = pv if t in (1, 2, 4) else pp
                    zt = pool.tile([c1 - c0, SB], F32, tag=f"z{t}")
                    for k in range(KT):
                        nc.tensor.matmul(
                            zt[:],
                            w_k[k][:, c0:c1],
                            x_sb[:, k, :],
                            start=(k == 0),
                            stop=(k == KT - 1),
                        )
                    z.append(zt)

                # DVE drains z0/z3 into the pair tiles (GPSIMD and DMA
                # cannot read PSUM; draining via Act sigmoids stalled PE
                # behind activation-table loads).  z1, z2, z4 are
                # double-buffered, giving their consumers a block of slack.
                nc.vector.tensor_copy(zc0[:, hs], z[0][:])
                nc.vector.tensor_copy(zc3[:, hs], z[3][:])

                # i = sigmoid(z1) per half so the u-muls can drain z2
                ii = sp.tile([128, SB], BF16, tag="ii")
                nc.scalar.activation(ii[:], z[1][:], AFT.Sigmoid)
                nc.vector.tensor_mul(u_lo[:, hs], ii[:], z[2][:])
                z4s.append(z[4])

            return dict(zc0=zc0, zc3=zc3, u_lo=u_lo, z4s=z4s)

        def emit_trail(m, st, prev_h):
            p0 = m * PW
            zc0, zc3, u_lo, z4s = st["zc0"], st["zc3"], st["u_lo"], st["z4s"]
            u_hi = sp.tile([64, PW], BF16, tag="u_hi")

            # --- pair-wide pointwise ----------------------------------
            # The chunk-start masking for the cd scan happens in the
            # PRE-activation domain: first tiny strided sigmoids capture
            # the true chunk-start a values (a_s), then the chunk-start
            # columns of zc0/zc3 are memset to -1e9 so the MAIN sigmoids
            # directly produce a with those columns zeroed (sig(-1e9)=0)
            # -- no full-width a_m copy on Pool or on the critical chain.
            as_lo = ap.tile([128, PW], F32, tag="as_lo")
            as_hi = ap.tile([64, PW], F32, tag="as_hi")
            if m < 2:
                nc.gpsimd.memset(as_lo[:], 0.0)
                nc.gpsimd.memset(as_hi[:], 0.0)
            nc.scalar.activation(as_lo[:, 0::CH], zc0[:, 0::CH],
                                 AFT.Sigmoid, bias=b0[:])
            nc.scalar.activation(as_hi[:, 0::CH], zc3[0:64, 0::CH],
                                 AFT.Sigmoid, bias=b3[0:64, :])
            # chunk-start masking IN-PLACE on the Act engine: emitted
            # between the a_s sigmoids (readers) and the main sigmoids
            # (readers), Act's in-order execution serializes them --
            # cross-engine strided-subtile WAR deps proved racy.
            nc.scalar.activation(zc0[:, 0::CH], zc0[:, 0::CH], AFT.Copy,
                                 bias=-1e9, scale=0.0)
            nc.scalar.activation(zc3[0:64, 0::CH], zc3[0:64, 0::CH],
                                 AFT.Copy, bias=-1e9, scale=0.0)

            a = sp.tile([128, PW], F32, tag="a")   # a_m: zeroed chunk starts
            nc.scalar.activation(a[:], zc0[:], AFT.Sigmoid, bias=b0[:])
            s3 = sp.tile([128, PW], F32, tag="s3")     # [a_hi_m ; i_hi]
            nc.scalar.activation(s3[:], zc3[:], AFT.Sigmoid, bias=b3[:])
            a_hi = s3[0:64, :]

            # realign i_hi (partitions 64:128) with v_hi (partitions 0:64)
            ic = sp.tile([64, PW], F32, tag="ic")
            nc.sync.dma_start(ic[:], s3[64:128, :])
            # u_hi per half (z4 tiles are double-buffered, both still live)
            for half in range(2):
                hs = slice(half * SB, (half + 1) * SB)
                nc.vector.tensor_mul(u_hi[:, hs], ic[:, hs], z4s[half][:])

            # squares + r = sqrt(1 - a^2) on Act (square/sqrt table ops;
            # the 1e-8 floor in the reference is unreachable).  The main
            # squares read the masked a; tiny strided squares from a_s
            # patch the chunk-start columns.
            msq = sp.tile([128, PW], F32, tag="msq")
            nc.scalar.activation(msq[:], a[:], AFT.Square)
            nc.scalar.activation(msq[:, 0::CH], as_lo[:, 0::CH], AFT.Square)
            m_hi = sp.tile([64, PW], F32, tag="m_hi")
            nc.scalar.activation(m_hi[:], a_hi, AFT.Square)
            nc.scalar.activation(m_hi[:, 0::CH], as_hi[:, 0::CH], AFT.Square)
            r_lo = sp.tile([128, PW], BF16, tag="r_lo")
            nc.scalar.activation(r_lo[:], msq[:], AFT.Sqrt, bias=1.0,
                                 scale=-1.0)
            r_hi = sp.tile([64, PW], BF16, tag="r_hi")
            nc.scalar.activation(r_hi[:], m_hi[:], AFT.Sqrt, bias=1.0,
                                 scale=-1.0)

            w_lo = sp.tile([128, PW], BF16, tag="w_lo")
            nc.vector.tensor_mul(w_lo[:], r_lo[:], u_lo[:])

            cd_lo = sp.tile([128, PW], F32, tag="cd_lo")
            nc.vector.tensor_tensor_scan(
                cd_lo[:], a[:], as_lo[:], 1.0, op0=OP.mult, op1=OP.add)
            cd_hi = sp.tile([64, PW], F32, tag="cd_hi")
            nc.vector.tensor_tensor_scan(
                cd_hi[:], a_hi, as_hi[:], 1.0, op0=OP.mult, op1=OP.add)

            # restore the true a at chunk starts for the h scan: on DVE,
            # so engine order guarantees cd-scan -> restore -> h-scan.
            nc.vector.tensor_copy(a[:, 0::CH], as_lo[:, 0::CH])
            nc.vector.tensor_copy(s3[0:64, 0::CH], as_hi[:, 0::CH])

            # g = min(cd*1e10, 1) as 1 - relu(1 - 1e10*cd) on Act
            t_lo = sp.tile([128, PW], BF16, tag="t_lo")
            nc.scalar.activation(t_lo[:], cd_lo[:], AFT.Relu, bias=1.0,
                                 scale=-1e10)
            g_lo = sp.tile([128, PW], BF16, tag="g_lo")
            nc.scalar.activation(g_lo[:], t_lo[:], AFT.Copy, bias=1.0,
                                 scale=-1.0)
            t_hi = sp.tile([64, PW], BF16, tag="t_hi")
            nc.scalar.activation(t_hi[:], cd_hi[:], AFT.Relu, bias=1.0,
                                 scale=-1e10)
            g_hi = sp.tile([64, PW], BF16, tag="g_hi")
            nc.scalar.activation(g_hi[:], t_hi[:], AFT.Copy, bias=1.0,
                                 scale=-1.0)

            gw_lo = sp.tile([128, PW], F32, tag="gw_lo")
            nc.vector.tensor_mul(gw_lo[:], g_lo[:], w_lo[:])

            h_lo = hp.tile([128, PW], F32, tag="h_lo")
            init_lo = 0.0 if prev_h is None else prev_h[0][:, PW - 1:PW]
            nc.vector.tensor_tensor_scan(
                h_lo[:], a[:], gw_lo[:], init_lo, op0=OP.mult, op1=OP.add)
            nc.sync.dma_start(out[0:128, p0:p0 + PW], h_lo[:])

            w_hi = sp.tile([64, PW], BF16, tag="w_hi")
            nc.vector.tensor_mul(w_hi[:], r_hi[:], u_hi[:])
            gw_hi = sp.tile([64, PW], F32, tag="gw_hi")
            nc.vector.tensor_mul(gw_hi[:], g_hi[:], w_hi[:])
            h_hi = hp.tile([64, PW], F32, tag="h_hi")
            init_hi = 0.0 if prev_h is None else prev_h[1][:, PW - 1:PW]
            nc.vector.tensor_tensor_scan(
                h_hi[:], a_hi, gw_hi[:], init_hi, op0=OP.mult, op1=OP.add)
            nc.sync.dma_start(out[128:DC, p0:p0 + PW], h_hi[:])
            return (h_lo, h_hi)

        # Software pipeline: emit pair m+1's matmuls + PSUM drains BEFORE
        # pair m's trailing pointwise, so the drains sit at the front of
        # the DVE queue and PE never stalls on z0/z3 release (measured
        # ~8us of stalls plus ~3us of post-stall half-clock matmuls).
        # Only z4 (last-issued tile, double-buffered) waits on trailing.
        prev_h = None
        sts = {}
        for m in range(NB // 2):
            sts[m] = emit_front(m)
            if m >= 1:
                prev_h = emit_trail(m - 1, sts.pop(m - 1), prev_h)
        last = NB // 2 - 1
        prev_h = emit_trail(last, sts.pop(last), prev_h)

    nc.finalize()
    return nc


def _make_in_maps(x, Wa, Wi, Wv, decay_bias):
    x = np.asarray(x, dtype=np.float32)
    Wa = np.asarray(Wa, dtype=np.float32)
    Wi = np.asarray(Wi, dtype=np.float32)
    Wv = np.asarray(Wv, dtype=np.float32)
    decay_bias = np.asarray(decay_bias, dtype=np.float32)

    in_maps = []
    for b in range(B):
        xTb = np.ascontiguousarray(x[b].T).astype(ml_dtypes.bfloat16)
        for j in range(2):
            c0 = j * DC
            wcat = np.concatenate(
                [
                    Wa[c0:c0 + 128].T,
                    Wi[c0:c0 + 128].T,
                    Wv[c0:c0 + 128].T,
                    Wa[c0 + 128:c0 + DC].T,
                    Wi[c0 + 128:c0 + DC].T,
                    Wv[c0 + 128:c0 + DC].T,
                ],
                axis=1,
            ).astype(ml_dtypes.bfloat16)
            b0 = np.ascontiguousarray(decay_bias[c0:c0 + 128, None])
            b3 = np.zeros((128, 1), dtype=np.float32)
            b3[0:64, 0] = decay_bias[c0 + 128:c0 + DC]
            in_maps.append({
                "xt": xTb,
                "wcat": np.ascontiguousarray(wcat),
                "bias0": b0,
                "bias3": b3,
            })
    return in_maps


def kernel(x, Wa, Wi, Wv, decay_bias):
    global _CACHED_NC
    if _CACHED_NC is None:
        _CACHED_NC = _build_nc()
    nc = _CACHED_NC

    in_maps = _make_in_maps(x, Wa, Wi, Wv, decay_bias)
    res = run_bass_kernel_spmd(nc, in_maps, core_ids=list(range(8)))

    out = np.empty((B, S, DR), dtype=np.float32)
    for b in range(B):
        for j in range(2):
            core = 2 * b + j
            out[b, :, j * DC:(j + 1) * DC] = res.results[core]["out"].T
    return out


# revision 24
# speedup vs baseline: 1.0170x; 1.0170x over previous
"""Trainium2 Bass kernel for the Griffin-style gated linear recurrence.

Model (matching the jax reference, including its chunked-scan numerics):
    a = sigmoid(x @ Wa.T + decay_bias)
    i = sigmoid(x @ Wi.T)
    v = x @ Wv.T
    w = sqrt(max(1 - a*a, 1e-8)) * i * v
    chunked scan (chunk=64), algebraically equal to
    h[t] = a[t]*h[t-1] + g[t]*w[t],  g[t] = min(1, cd[t]*1e10),
    cd = within-chunk running product of a (reset every 64 steps).

Sharding: 4 batches x 2 channel-halves = 8 cores, no communication.

Matmul layout (per core): channels on partitions, time on the free axis.
The 576 projection columns (3 gates x 192 channels) are packed into 5
stationary tiles per k-tile instead of 6:
    z0 = a[0:128]   z1 = i[0:128]   z2 = v[0:128]
    z3 = [a[128:192] ; i[128:192]]  (64+64 partitions, one sigmoid with a
                                     [bias_hi ; 0] per-partition bias)
    z4 = v[128:192]                  (64 partitions)
which cuts tensor-engine time by 1/6 (matmul cost is N-cycles per
instruction regardless of M).  x and weights stream in bf16; PSUM
accumulates fp32.  A dozen warm-up matmuls on a zeroed scratch tile
bring the PE out of its low p-state (0.65->2.4GHz takes ~3us of
continuous busy) while the first x block is still in flight.

Pipeline structure (learned from several traced revisions):
  * Matmuls/PSUM work per 512-column block, but ALL trailing pointwise
    runs once per 1024-column block pair -- per-instruction overheads on
    DVE/Pool/Act were measured at 0.4-1.3us, so halving the op count is
    worth more than any engine rebalancing.
  * PSUM z0/z3 (bufs=1) are drained by DVE copies into SBUF pair tiles
    (GPSIMD and DMA cannot read PSUM) and the sigmoids read those
    copies.  Draining via the sigmoids directly made PE stall up to
    8.4us per block: the Act queue sits behind 1.28us activation-table
    loads (sigmoid and sqrt never share a table) and a stalled PE also
    drops back to half clock for ~3us.
  * z1/z2/z4 are double-buffered (2 + 3*2 = 8 PSUM banks, the warm-up
    sharing a z2 slot) and consumed directly at 512 width (Act ii
    sigmoid, DVE u-muls).
  * g = min(cd*1e10, 1) runs on Act as t=Relu(1-1e10*cd); g=1-t (relu
    and copy live in EVERY act table; tensor_scalar was 7.5us/op on
    Pool's Q7 path and the DVE needs the slack for the scans).
  * cd is ONE masked scan per group pair, with the chunk-start masking
    done in the PRE-activation domain: tiny strided sigmoids capture the
    true chunk-start a values (a_s), the chunk-start columns of the
    drained z are memset to -1e9, and the main sigmoids then directly
    produce a with zeroed chunk starts -- the scan self-resets, and no
    full-width a_m copy exists on any engine or on the critical chain.
    Strided fix-ups patch m at those columns and restore true a for the
    h scan afterwards.
  * The decay path (a, m, cd, h, scan operands) stays fp32: sqrt(1-a^2)
    is cancellation-sensitive for slow channels, and a bf16 scan operand
    was measured to double scan time.  The iv path runs bf16.
  * Moving the squares or the gw muls to Pool was measured SLOWER
    (139.9us vs 114.5us): Pool's Q7 tensor_tensor (~3.2us per 1024-wide
    op) sits on the gw->h critical chain.

z3 holds a_hi/i_hi on different partitions, so one cross-partition
SBUF->SBUF DMA per pair realigns i_hi with v_hi (engines are
lane-locked; only DMA can move data across partitions).
"""

import sys

if "/opt/trn_rl_repo" not in sys.path:
    sys.path.insert(0, "/opt/trn_rl_repo")

from contextlib import ExitStack

import numpy as np
import ml_dtypes

from concourse import bacc, bass, mybir, tile
from concourse.bass_utils import run_bass_kernel_spmd

B, S = 4, 4096
DM, DR = 1024, 384
DC = DR // 2          # channels per core
CH = 64               # scan chunk size
SB = 512              # matmul/PSUM block
PW = 2 * SB           # pointwise pair width
NB = S // SB
KT = DM // 128        # contraction tiles

F32 = mybir.dt.float32
BF16 = mybir.dt.bfloat16
AFT = mybir.ActivationFunctionType
OP = mybir.AluOpType

# column ranges of the 5 packed stationary tiles
TCOLS = ((0, 128), (128, 256), (256, 384), (384, 512), (512, 576))

_CACHED_NC = None


def _build_nc():
    nc = bacc.Bacc(trn_type="TRN2")

    xT = nc.dram_tensor("xt", [DM, S], BF16, kind="ExternalInput")
    wc = nc.dram_tensor("wcat", [DM, 576], BF16, kind="ExternalInput")
    bias0 = nc.dram_tensor("bias0", [128, 1], F32, kind="ExternalInput")
    bias3 = nc.dram_tensor("bias3", [128, 1], F32, kind="ExternalInput")
    out = nc.dram_tensor("out", [DC, S], F32, kind="ExternalOutput")

    with tile.TileContext(nc) as tc, ExitStack() as ctx:
        wp = ctx.enter_context(tc.tile_pool(name="wp", bufs=1))
        cp = ctx.enter_context(tc.tile_pool(name="cp", bufs=1))
        xp = ctx.enter_context(tc.tile_pool(name="xp", bufs=2))
        pp = ctx.enter_context(tc.tile_pool(name="pp", bufs=1, space="PSUM"))
        pv = ctx.enter_context(tc.tile_pool(name="pv", bufs=2, space="PSUM"))
        sp = ctx.enter_context(tc.tile_pool(name="sp", bufs=2))
        ap = ctx.enter_context(tc.tile_pool(name="ap", bufs=2))
        hp = ctx.enter_context(tc.tile_pool(name="hp", bufs=2))

        # --- warm-up + constants --------------------------------------
        # x block 0 first: it gates the first real matmul.
        x0 = xp.tile([128, KT, SB], BF16, tag="x")
        nc.sync.dma_start(
            x0[:], xT.rearrange("(k p) s -> p k s", p=128)[:, :, 0:SB])

        # PE warm-up: a zeroed bf16 tile matmul'd into the spare PSUM
        # bank while DMAs land; PE reaches full clock before real work.
        wz = cp.tile([128, SB], BF16, tag="wz")
        nc.gpsimd.memset(wz[:], 0.0)
        zw = pv.tile([128, SB], F32, tag="z2")  # reuses a z2 PSUM slot
        for _ in range(12):
            nc.tensor.matmul(zw[:], wz[:, 0:128], wz[:], start=True,
                             stop=True)

        # per-k weight tiles so the first matmuls gate on 144KB, not 1.15MB
        w_k = []
        for k in range(KT):
            wk = wp.tile([128, 576], BF16, tag=f"wk{k}")
            nc.sync.dma_start(wk[:], wc[k * 128:(k + 1) * 128, :])
            w_k.append(wk)

        b0 = cp.tile([128, 1], F32, tag="b0")
        nc.scalar.dma_start(b0[:], bias0[:, :])
        b3 = cp.tile([128, 1], F32, tag="b3")
        nc.scalar.dma_start(b3[:], bias3[:, :])

        prev_h = None
        for m in range(NB // 2):
            p0 = m * PW

            # SBUF pair tiles that the two matmul sub-blocks fill
            zc0 = sp.tile([128, PW], F32, tag="zc0")
            zc3 = sp.tile([128, PW], F32, tag="zc3")
            u_lo = sp.tile([128, PW], BF16, tag="u_lo")
            u_hi = sp.tile([64, PW], BF16, tag="u_hi")

            z4s = []
            for half in range(2):
                ib = 2 * m + half
                s0 = ib * SB
                hs = slice(half * SB, (half + 1) * SB)

                if ib > 0:  # block 0's x tile was loaded up front
                    x_sb = xp.tile([128, KT, SB], BF16, tag="x")
                    nc.sync.dma_start(
                        x_sb[:],
                        xT.rearrange("(k p) s -> p k s", p=128)
                        [:, :, s0:s0 + SB])
                else:
                    x_sb = x0

                z = []
                for t, (c0, c1) in enumerate(TCOLS):
                    pool = pv if t in (1, 2, 4) else pp
                    zt = pool.tile([c1 - c0, SB], F32, tag=f"z{t}")
                    for k in range(KT):
                        nc.tensor.matmul(
                            zt[:],
                            w_k[k][:, c0:c1],
                            x_sb[:, k, :],
                            start=(k == 0),
                            stop=(k == KT - 1),
                        )
                    z.append(zt)

                # DVE drains z0/z3 into the pair tiles (GPSIMD and DMA
                # cannot read PSUM; draining via Act sigmoids stalled PE
                # behind activation-table loads).  z1, z2, z4 are
                # double-buffered, giving their consumers a block of slack.
                nc.vector.tensor_copy(zc0[:, hs], z[0][:])
                nc.vector.tensor_copy(zc3[:, hs], z[3][:])

                # i = sigmoid(z1) per half so the u-muls can drain z2
                ii = sp.tile([128, SB], BF16, tag="ii")
                nc.scalar.activation(ii[:], z[1][:], AFT.Sigmoid)
                nc.vector.tensor_mul(u_lo[:, hs], ii[:], z[2][:])
                z4s.append(z[4])

            # --- pair-wide pointwise ----------------------------------
            # The chunk-start masking for the cd scan happens in the
            # PRE-activation domain: first tiny strided sigmoids capture
            # the true chunk-start a values (a_s), then the chunk-start
            # columns of zc0/zc3 are memset to -1e9 so the MAIN sigmoids
            # directly produce a with those columns zeroed (sig(-1e9)=0)
            # -- no full-width a_m copy on Pool or on the critical chain.
            as_lo = ap.tile([128, PW], F32, tag="as_lo")
            as_hi = ap.tile([64, PW], F32, tag="as_hi")
            if m < 2:
                nc.gpsimd.memset(as_lo[:], 0.0)
                nc.gpsimd.memset(as_hi[:], 0.0)
            nc.scalar.activation(as_lo[:, 0::CH], zc0[:, 0::CH],
                                 AFT.Sigmoid, bias=b0[:])
            nc.scalar.activation(as_hi[:, 0::CH], zc3[0:64, 0::CH],
                                 AFT.Sigmoid, bias=b3[0:64, :])
            # chunk-start masking IN-PLACE on the Act engine: emitted
            # between the a_s sigmoids (readers) and the main sigmoids
            # (readers), Act's in-order execution serializes them --
            # cross-engine strided-subtile WAR deps proved racy.
            nc.scalar.activation(zc0[:, 0::CH], zc0[:, 0::CH], AFT.Copy,
                                 bias=-1e9, scale=0.0)
            nc.scalar.activation(zc3[0:64, 0::CH], zc3[0:64, 0::CH],
                                 AFT.Copy, bias=-1e9, scale=0.0)

            a = sp.tile([128, PW], F32, tag="a")   # a_m: zeroed chunk starts
            nc.scalar.activation(a[:], zc0[:], AFT.Sigmoid, bias=b0[:])
            s3 = sp.tile([128, PW], F32, tag="s3")     # [a_hi_m ; i_hi]
            nc.scalar.activation(s3[:], zc3[:], AFT.Sigmoid, bias=b3[:])
            a_hi = s3[0:64, :]

            # realign i_hi (partitions 64:128) with v_hi (partitions 0:64)
            ic = sp.tile([64, PW], F32, tag="ic")
            nc.sync.dma_start(ic[:], s3[64:128, :])
            # u_hi per half (z4 tiles are double-buffered, both still live)
            for half in range(2):
                hs = slice(half * SB, (half + 1) * SB)
                nc.vector.tensor_mul(u_hi[:, hs], ic[:, hs], z4s[half][:])

            # squares + r = sqrt(1 - a^2) on Act (square/sqrt table ops;
            # the 1e-8 floor in the reference is unreachable).  The main
            # squares read the masked a; tiny strided squares from a_s
            # patch the chunk-start columns.
            msq = sp.tile([128, PW], F32, tag="msq")
            nc.scalar.activation(msq[:], a[:], AFT.Square)
            nc.scalar.activation(msq[:, 0::CH], as_lo[:, 0::CH], AFT.Square)
            m_hi = sp.tile([64, PW], F32, tag="m_hi")
            nc.scalar.activation(m_hi[:], a_hi, AFT.Square)
            nc.scalar.activation(m_hi[:, 0::CH], as_hi[:, 0::CH], AFT.Square)
            r_lo = sp.tile([128, PW], BF16, tag="r_lo")
            nc.scalar.activation(r_lo[:], msq[:], AFT.Sqrt, bias=1.0,
                                 scale=-1.0)
            r_hi = sp.tile([64, PW], BF16, tag="r_hi")
            nc.scalar.activation(r_hi[:], m_hi[:], AFT.Sqrt, bias=1.0,
                                 scale=-1.0)

            w_lo = sp.tile([128, PW], BF16, tag="w_lo")
            nc.vector.tensor_mul(w_lo[:], r_lo[:], u_lo[:])

            cd_lo = sp.tile([128, PW], F32, tag="cd_lo")
            nc.vector.tensor_tensor_scan(
                cd_lo[:], a[:], as_lo[:], 1.0, op0=OP.mult, op1=OP.add)
            cd_hi = sp.tile([64, PW], F32, tag="cd_hi")
            nc.vector.tensor_tensor_scan(
                cd_hi[:], a_hi, as_hi[:], 1.0, op0=OP.mult, op1=OP.add)

            # restore the true a at chunk starts for the h scan: on DVE,
            # so engine order guarantees cd-scan -> restore -> h-scan.
            nc.vector.tensor_copy(a[:, 0::CH], as_lo[:, 0::CH])
            nc.vector.tensor_copy(s3[0:64, 0::CH], as_hi[:, 0::CH])

            # g = min(cd*1e10, 1) as 1 - relu(1 - 1e10*cd) on Act
            t_lo = sp.tile([128, PW], BF16, tag="t_lo")
            nc.scalar.activation(t_lo[:], cd_lo[:], AFT.Relu, bias=1.0,
                                 scale=-1e10)
            g_lo = sp.tile([128, PW], BF16, tag="g_lo")
            nc.scalar.activation(g_lo[:], t_lo[:], AFT.Copy, bias=1.0,
                                 scale=-1.0)
            t_hi = sp.tile([64, PW], BF16, tag="t_hi")
            nc.scalar.activation(t_hi[:], cd_hi[:], AFT.Relu, bias=1.0,
                                 scale=-1e10)
            g_hi = sp.tile([64, PW], BF16, tag="g_hi")
            nc.scalar.activation(g_hi[:], t_hi[:], AFT.Copy, bias=1.0,
                                 scale=-1.0)

            gw_lo = sp.tile([128, PW], F32, tag="gw_lo")
            nc.vector.tensor_mul(gw_lo[:], g_lo[:], w_lo[:])

            h_lo = hp.tile([128, PW], F32, tag="h_lo")
            init_lo = 0.0 if prev_h is None else prev_h[0][:, PW - 1:PW]
            nc.vector.tensor_tensor_scan(
                h_lo[:], a[:], gw_lo[:], init_lo, op0=OP.mult, op1=OP.add)
            nc.sync.dma_start(out[0:128, p0:p0 + PW], h_lo[:])

            w_hi = sp.tile([64, PW], BF16, tag="w_hi")
            nc.vector.tensor_mul(w_hi[:], r_hi[:], u_hi[:])
            gw_hi = sp.tile([64, PW], F32, tag="gw_hi")
            nc.vector.tensor_mul(gw_hi[:], g_hi[:], w_hi[:])
            h_hi = hp.tile([64, PW], F32, tag="h_hi")
            init_hi = 0.0 if prev_h is None else prev_h[1][:, PW - 1:PW]
            nc.vector.tensor_tensor_scan(
                h_hi[:], a_hi, gw_hi[:], init_hi, op0=OP.mult, op1=OP.add)
            nc.sync.dma_start(out[128:DC, p0:p0 + PW], h_hi[:])

            prev_h = (h_lo, h_hi)

    nc.finalize()
    return nc


def _make_in_maps(x, Wa, Wi, Wv, decay_bias):
    x = np.asarray(x, dtype=np.float32)
    Wa = np.asarray(Wa, dtype=np.float32)
    Wi = np.asarray(Wi, dtype=np.float32)
    Wv = np.asarray(Wv, dtype=np.float32)
    decay_bias = np.asarray(decay_bias, dtype=np.float32)

    in_maps = []
    for b in range(B):
        xTb = np.ascontiguousarray(x[b].T).astype(ml_dtypes.bfloat16)
        for j in range(2):
            c0 = j * DC
            wcat = np.concatenate(
                [
                    Wa[c0:c0 + 128].T,
                    Wi[c0:c0 + 128].T,
                    Wv[c0:c0 + 128].T,
                    Wa[c0 + 128:c0 + DC].T,
                    Wi[c0 + 128:c0 + DC].T,
                    Wv[c0 + 128:c0 + DC].T,
                ],
                axis=1,
            ).astype(ml_dtypes.bfloat16)
            b0 = np.ascontiguousarray(decay_bias[c0:c0 + 128, None])
            b3 = np.zeros((128, 1), dtype=np.float32)
            b3[0:64, 0] = decay_bias[c0 + 128:c0 + DC]
            in_maps.append({
                "xt": xTb,
                "wcat": np.ascontiguousarray(wcat),
                "bias0": b0,
                "bias3": b3,
            })
    return in_maps


def kernel(x, Wa, Wi, Wv, decay_bias):
    global _CACHED_NC
    if _CACHED_NC is None:
        _CACHED_NC = _build_nc()
    nc = _CACHED_NC

    in_maps = _make_in_maps(x, Wa, Wi, Wv, decay_bias)
    res = run_bass_kernel_spmd(nc, in_maps, core_ids=list(range(8)))

    out = np.empty((B, S, DR), dtype=np.float32)
    for b in range(B):
        for j in range(2):
            core = 2 * b + j
            out[b, :, j * DC:(j + 1) * DC] = res.results[core]["out"].T
    return out
